# revision 7
# baseline (speedup 1.0000x reference)
"""HalfEdgeConv on 8 TRN2 NeuronCores.

out = relu(concat_k x[neighbor_idx[:, k]] @ W + b)

Strategy: data-parallel over the half-edge dim E. Each core gets the full x
table (random neighbor indices span all of E, so a halo exchange would be
all-to-all anyway), a 1/8 shard of neighbor_idx, and replicated W/b.

Per-core kernel:
  - indirect DMA (SWDGE) gathers the 5 neighbor rows per edge straight into
    SBUF as [128 edges, 5*64] concatenated features, J tiles per instruction
  - PE transposes feature chunks ([128e, cw] -> [cw, 128e]) via identity
    matmul, DVE copies PSUM->SBUF
  - 3 accumulating matmuls (K chunks 128+128+64) with W stationary produce
    out.T = W.T @ feats.T in PSUM [64co, 128e]
  - ACT applies relu(psum + b) with per-partition bias, output staged
    transposed ([64, E_pad]) so output DMA writes are 8KB-contiguous per
    partition; the host undoes the transpose.

Host-side prep: int64->int32 index cast, per-tile index permutation
([E,5] -> [128, T*5] so partition p holds tile-local edge p's indices),
W padded into [128, 3*64] K-chunks, bias as a [64,1] column.
"""

from contextlib import ExitStack

import numpy as np

import concourse.bass as bass
import concourse.tile as tile
from concourse import bacc, mybir
from concourse.bass import IndirectOffsetOnAxis
from concourse.bass_utils import run_bass_kernel_spmd
from concourse.masks import make_identity

E = 1_000_000
KN = 5
C_IN = 64
C_OUT = 64
N_CORES = 8
P = 128
E_SHARD = E // N_CORES          # 125_000
T = -(-E_SHARD // P)            # 977 tiles of 128 edges
E_PAD = T * P                   # 125_056
J = 16                          # tiles per gather block
F = KN * C_IN                   # 320 concatenated features
W_COLS = 3 * C_OUT              # K chunks 128+128+64 stored as [128, 192]

_f32 = mybir.dt.float32
_f32r = mybir.dt.float32r
_i32 = mybir.dt.int32


def build_program(e_tab: int = E, n_tiles: int = T, j_blk: int = J) -> bacc.Bacc:
    nc = bacc.Bacc(
        "TRN2", target_bir_lowering=False, debug=False, num_devices=N_CORES
    )
    x_d = nc.dram_tensor("x", [e_tab, C_IN], _f32, kind="ExternalInput").ap()
    idx_d = nc.dram_tensor("idx", [P, n_tiles * KN], _i32, kind="ExternalInput").ap()
    w_d = nc.dram_tensor("w", [P, W_COLS], _f32, kind="ExternalInput").ap()
    b_d = nc.dram_tensor("b", [C_OUT, 1], _f32, kind="ExternalInput").ap()
    o_d = nc.dram_tensor("o", [C_OUT, n_tiles * P], _f32, kind="ExternalOutput").ap()

    blocks = [(i * j_blk, j_blk) for i in range(n_tiles // j_blk)]
    if n_tiles % j_blk:
        blocks.append((n_tiles - n_tiles % j_blk, n_tiles % j_blk))

    relu = mybir.ActivationFunctionType.Relu

    with tile.TileContext(nc) as tc, ExitStack() as ctx:
        const_p = ctx.enter_context(tc.tile_pool(name="const", bufs=1))
        ident = const_p.tile([P, P], _f32)
        make_identity(nc, ident[:])
        w_sb = const_p.tile([P, W_COLS], _f32)
        nc.sync.dma_start(out=w_sb[:], in_=w_d[:])
        # walrus requires f32r matmul operands to be produced as f32r
        # (rounded); a one-time DVE copy does the rounding cast for W
        w_sbr = const_p.tile([P, W_COLS], _f32r)
        nc.vector.tensor_copy(out=w_sbr[:], in_=w_sb[:])
        b_sb = const_p.tile([C_OUT, 1], _f32)
        nc.sync.dma_start(out=b_sb[:], in_=b_d[:])

        idx_p = ctx.enter_context(tc.tile_pool(name="idx", bufs=10))
        g_p = ctx.enter_context(tc.tile_pool(name="g", bufs=4))
        ft_p = ctx.enter_context(tc.tile_pool(name="ft", bufs=4))
        o_p = ctx.enter_context(tc.tile_pool(name="o", bufs=3))
        tp_p = ctx.enter_context(tc.tile_pool(name="tp", bufs=4, space="PSUM"))
        po_p = ctx.enter_context(tc.tile_pool(name="po", bufs=2, space="PSUM"))

        for t0, jb in blocks:
            idx_sb = idx_p.tile([P, jb * KN], _i32, tag="idx")
            nc.sync.dma_start(
                out=idx_sb[:], in_=idx_d[:, t0 * KN : (t0 + jb) * KN]
            )
            g = g_p.tile([P, jb * F], _f32, tag="g")
            # HW contract for indirect DMA: ONE index per dest partition,
            # fetching the dest partition's contiguous run from that row.
            # So one gather per (tile, k): dest [128, 64] slice, idx [128, 1].
            for j in range(jb):
                for k in range(KN):
                    col = j * KN + k
                    nc.gpsimd.indirect_dma_start(
                        out=g[:, col * C_IN : (col + 1) * C_IN],
                        out_offset=None,
                        in_=x_d[:],
                        in_offset=IndirectOffsetOnAxis(
                            ap=idx_sb[:, col : col + 1], axis=0
                        ),
                    )
            o_sb = o_p.tile([C_OUT, jb * P], _f32, tag="o")
            # groups of up to 4 tiles -> moving dim 512 so f32r matmuls run
            # at 1 cycle/row (fp32 is 4 cy/row, f32r below N=256 also 4)
            for g0 in range(0, jb, 4):
                gw = min(4, jb - g0)          # tiles in this group
                n_mov = gw * P                # matmul moving dim
                po = po_p.tile([C_OUT, n_mov], _f32, tag="po")
                for c in range(3):
                    cw = 128 if c < 2 else 64
                    ft = ft_p.tile([P, n_mov], _f32r, tag="ft")
                    for j2 in range(gw):
                        f0 = (g0 + j2) * F + c * 128
                        tp = tp_p.tile([P, P], _f32, tag="tp")
                        nc.tensor.transpose(
                            out=tp[:cw, :], in_=g[:, f0 : f0 + cw],
                            identity=ident[:],
                        )
                        nc.vector.tensor_copy(
                            out=ft[:cw, j2 * P : (j2 + 1) * P], in_=tp[:cw, :]
                        )
                    nc.tensor.matmul(
                        out=po[:],
                        lhsT=w_sbr[:cw, c * C_OUT : (c + 1) * C_OUT],
                        rhs=ft[:cw, :],
                        start=(c == 0),
                        stop=(c == 2),
                    )
                nc.scalar.activation(
                    out=o_sb[:, g0 * P : g0 * P + n_mov], in_=po[:], func=relu,
                    bias=b_sb[:],
                )
            nc.sync.dma_start(
                out=o_d[:, t0 * P : (t0 + jb) * P], in_=o_sb[:]
            )
    nc.compile()
    return nc


def make_in_maps(x, neighbor_idx, W, b):
    """Host-side shard + layout prep. Returns per-core input dicts."""
    x = np.ascontiguousarray(np.asarray(x), dtype=np.float32)
    idx = np.asarray(neighbor_idx)
    W = np.asarray(W, dtype=np.float32)
    b = np.asarray(b, dtype=np.float32)

    w_pad = np.zeros((P, W_COLS), np.float32)
    w_pad[:, 0:C_OUT] = W[0:128]
    w_pad[:, C_OUT : 2 * C_OUT] = W[128:256]
    w_pad[:64, 2 * C_OUT : 3 * C_OUT] = W[256:320]
    b_col = np.ascontiguousarray(b.reshape(C_OUT, 1))

    in_maps = []
    for c in range(N_CORES):
        sh = idx[c * E_SHARD : (c + 1) * E_SHARD].astype(np.int32)
        sh_pad = np.zeros((E_PAD, KN), np.int32)
        sh_pad[:E_SHARD] = sh
        idx_perm = np.ascontiguousarray(
            sh_pad.reshape(T, P, KN).transpose(1, 0, 2)
        ).reshape(P, T * KN)
        in_maps.append({"x": x, "idx": idx_perm, "w": w_pad, "b": b_col})
    return in_maps


def unshard_output(per_core_out):
    """[64, E_PAD] per core -> full [E, 64]."""
    outs = [op.T[:E_SHARD] for op in per_core_out]
    return np.ascontiguousarray(np.concatenate(outs, axis=0), dtype=np.float32)


_program_cache: dict[str, bacc.Bacc] = {}


def get_program() -> bacc.Bacc:
    if "nc" not in _program_cache:
        _program_cache["nc"] = build_program()
    return _program_cache["nc"]


def kernel(x, neighbor_idx, W, b):
    nc = get_program()
    in_maps = make_in_maps(x, neighbor_idx, W, b)
    res = run_bass_kernel_spmd(nc, in_maps, list(range(N_CORES)))
    return unshard_output([res.results[c]["o"] for c in range(N_CORES)])


# revision 11
# speedup vs baseline: 15.3991x; 15.3991x over previous
"""HalfEdgeConv on 8 TRN2 NeuronCores.

out = relu(concat_k x[neighbor_idx[:, k]] @ W + b)

Strategy: data-parallel over the half-edge dim E. Each core gets the full x
table (random neighbor indices span all of E, so a halo exchange would be
all-to-all anyway), a 1/8 shard of neighbor_idx, and replicated W/b.

Per-core kernel:
  - indirect DMA (SWDGE) gathers the 5 neighbor rows per edge straight into
    SBUF as [128 edges, 5*64] concatenated features, J tiles per instruction
  - PE transposes feature chunks ([128e, cw] -> [cw, 128e]) via identity
    matmul, DVE copies PSUM->SBUF
  - 3 accumulating matmuls (K chunks 128+128+64) with W stationary produce
    out.T = W.T @ feats.T in PSUM [64co, 128e]
  - ACT applies relu(psum + b) with per-partition bias, output staged
    transposed ([64, E_pad]) so output DMA writes are 8KB-contiguous per
    partition; the host undoes the transpose.

Host-side prep: int64->int32 index cast, per-tile index permutation
([E,5] -> [128, T*5] so partition p holds tile-local edge p's indices),
W padded into [128, 3*64] K-chunks, bias as a [64,1] column.
"""

from contextlib import ExitStack

import numpy as np

import concourse.bass as bass
import concourse.tile as tile
from concourse import bacc, mybir
from concourse.bass import IndirectOffsetOnAxis
from concourse.bass_utils import run_bass_kernel_spmd
from concourse.masks import make_identity

E = 1_000_000
KN = 5
C_IN = 64
C_OUT = 64
N_CORES = 8
P = 128
E_SHARD = E // N_CORES          # 125_000
T = -(-E_SHARD // P)            # 977 tiles of 128 edges
E_PAD = T * P                   # 125_056
J = 16                          # tiles per gather block
F = KN * C_IN                   # 320 concatenated features
W_COLS = 3 * C_OUT              # K chunks 128+128+64 stored as [128, 192]

_f32 = mybir.dt.float32
_f32r = mybir.dt.float32r
_i32 = mybir.dt.int32


def build_program(e_tab: int = E, n_tiles: int = T, j_blk: int = J,
                  skip_out_dma: bool = False, out_engine: str = "sync",
                  gather_only: bool = False, g_bufs: int = 6) -> bacc.Bacc:
    nc = bacc.Bacc(
        "TRN2", target_bir_lowering=False, debug=False, num_devices=N_CORES
    )
    x_d = nc.dram_tensor("x", [e_tab, C_IN], _f32, kind="ExternalInput").ap()
    idx_d = nc.dram_tensor("idx", [P, n_tiles * KN], _i32, kind="ExternalInput").ap()
    w_d = nc.dram_tensor("w", [P, W_COLS], _f32, kind="ExternalInput").ap()
    b_d = nc.dram_tensor("b", [C_OUT, 1], _f32, kind="ExternalInput").ap()
    o_d = nc.dram_tensor("o", [C_OUT, n_tiles * P], _f32, kind="ExternalOutput").ap()

    blocks = [(i * j_blk, j_blk) for i in range(n_tiles // j_blk)]
    if n_tiles % j_blk:
        blocks.append((n_tiles - n_tiles % j_blk, n_tiles % j_blk))

    relu = mybir.ActivationFunctionType.Relu

    with tile.TileContext(nc) as tc, ExitStack() as ctx:
        const_p = ctx.enter_context(tc.tile_pool(name="const", bufs=1))
        ident = const_p.tile([P, P], _f32)
        make_identity(nc, ident[:])
        w_sb = const_p.tile([P, W_COLS], _f32)
        nc.sync.dma_start(out=w_sb[:], in_=w_d[:])
        # walrus requires f32r matmul operands to be produced as f32r
        # (rounded); a one-time DVE copy does the rounding cast for W
        w_sbr = const_p.tile([P, W_COLS], _f32r)
        nc.vector.tensor_copy(out=w_sbr[:], in_=w_sb[:])
        b_sb = const_p.tile([C_OUT, 1], _f32)
        nc.sync.dma_start(out=b_sb[:], in_=b_d[:])

        idx_p = ctx.enter_context(tc.tile_pool(name="idx", bufs=10))
        g_p = ctx.enter_context(tc.tile_pool(name="g", bufs=g_bufs))
        ft_p = ctx.enter_context(tc.tile_pool(name="ft", bufs=4))
        o_p = ctx.enter_context(tc.tile_pool(name="o", bufs=3))
        tp_p = ctx.enter_context(tc.tile_pool(name="tp", bufs=4, space="PSUM"))
        po_p = ctx.enter_context(tc.tile_pool(name="po", bufs=2, space="PSUM"))

        for t0, jb in blocks:
            idx_sb = idx_p.tile([P, jb * KN], _i32, tag="idx")
            nc.sync.dma_start(
                out=idx_sb[:], in_=idx_d[:, t0 * KN : (t0 + jb) * KN]
            )
            g = g_p.tile([P, jb * F], _f32, tag="g")
            # HW contract for indirect DMA: ONE index per dest partition,
            # fetching the dest partition's contiguous run from that row.
            # So one gather per (tile, k): dest [128, 64] slice, idx [128, 1].
            for j in range(jb):
                for k in range(KN):
                    col = j * KN + k
                    nc.gpsimd.indirect_dma_start(
                        out=g[:, col * C_IN : (col + 1) * C_IN],
                        out_offset=None,
                        in_=x_d[:],
                        in_offset=IndirectOffsetOnAxis(
                            ap=idx_sb[:, col : col + 1], axis=0
                        ),
                    )
            if gather_only:
                continue
            o_sb = o_p.tile([C_OUT, jb * P], _f32, tag="o")
            # groups of up to 4 tiles -> moving dim 512 so f32r matmuls run
            # at 1 cycle/row (fp32 is 4 cy/row, f32r below N=256 also 4)
            for g0 in range(0, jb, 4):
                gw = min(4, jb - g0)          # tiles in this group
                n_mov = gw * P                # matmul moving dim
                po = po_p.tile([C_OUT, n_mov], _f32, tag="po")
                for c in range(3):
                    cw = 128 if c < 2 else 64
                    ft = ft_p.tile([P, n_mov], _f32r, tag="ft")
                    for j2 in range(gw):
                        f0 = (g0 + j2) * F + c * 128
                        tp = tp_p.tile([P, P], _f32, tag="tp")
                        nc.tensor.transpose(
                            out=tp[:cw, :], in_=g[:, f0 : f0 + cw],
                            identity=ident[:],
                        )
                        nc.vector.tensor_copy(
                            out=ft[:cw, j2 * P : (j2 + 1) * P], in_=tp[:cw, :]
                        )
                    nc.tensor.matmul(
                        out=po[:],
                        lhsT=w_sbr[:cw, c * C_OUT : (c + 1) * C_OUT],
                        rhs=ft[:cw, :],
                        start=(c == 0),
                        stop=(c == 2),
                    )
                nc.scalar.activation(
                    out=o_sb[:, g0 * P : g0 * P + n_mov], in_=po[:], func=relu,
                    bias=b_sb[:],
                )
            if not skip_out_dma:
                eng = nc.scalar if out_engine == "scalar" else nc.sync
                eng.dma_start(
                    out=o_d[:, t0 * P : (t0 + jb) * P], in_=o_sb[:]
                )
    nc.compile()
    return nc


def make_in_maps(x, neighbor_idx, W, b):
    """Host-side shard + layout prep. Returns per-core input dicts."""
    x = np.ascontiguousarray(np.asarray(x), dtype=np.float32)
    idx = np.asarray(neighbor_idx)
    W = np.asarray(W, dtype=np.float32)
    b = np.asarray(b, dtype=np.float32)

    w_pad = np.zeros((P, W_COLS), np.float32)
    w_pad[:, 0:C_OUT] = W[0:128]
    w_pad[:, C_OUT : 2 * C_OUT] = W[128:256]
    w_pad[:64, 2 * C_OUT : 3 * C_OUT] = W[256:320]
    b_col = np.ascontiguousarray(b.reshape(C_OUT, 1))

    in_maps = []
    for c in range(N_CORES):
        sh = idx[c * E_SHARD : (c + 1) * E_SHARD].astype(np.int32)
        sh_pad = np.zeros((E_PAD, KN), np.int32)
        sh_pad[:E_SHARD] = sh
        idx_perm = np.ascontiguousarray(
            sh_pad.reshape(T, P, KN).transpose(1, 0, 2)
        ).reshape(P, T * KN)
        in_maps.append({"x": x, "idx": idx_perm, "w": w_pad, "b": b_col})
    return in_maps


def unshard_output(per_core_out):
    """[64, E_PAD] per core -> full [E, 64]."""
    outs = [op.T[:E_SHARD] for op in per_core_out]
    return np.ascontiguousarray(np.concatenate(outs, axis=0), dtype=np.float32)


_program_cache: dict[str, bacc.Bacc] = {}


def get_program() -> bacc.Bacc:
    if "nc" not in _program_cache:
        _program_cache["nc"] = build_program()
    return _program_cache["nc"]


def kernel(x, neighbor_idx, W, b):
    nc = get_program()
    in_maps = make_in_maps(x, neighbor_idx, W, b)
    res = run_bass_kernel_spmd(nc, in_maps, list(range(N_CORES)))
    return unshard_output([res.results[c]["o"] for c in range(N_CORES)])
